# revision 1
# baseline (speedup 1.0000x reference)
"""AtomAttentionDecoder — 8-core sharded kernel (batch x sequence-half).

Sharding per the hint: data-parallel over batch (B=4) x sequence-parallel over
the atom axis (2 halves of 8192) = 8 shards, one per NeuronCore. Attention is
local (128-key window), so each shard carries a 256-atom halo per side; halo
atoms are recomputed locally (3 blocks consume at most 3*80 = 240 < 256 halo
atoms), so shards are fully independent — no collectives. Small weights are
replicated (baked into the NEFF as constants). The host does the cheap
token->atom gather and the tiny pair-bias MLP; the 8 NeuronCores run the 3
transformer blocks + output projection (>97% of the FLOPs) as 8 async jit
dispatches of one shared executable. Falls back to pure NumPy if the neuron
backend is unavailable.
"""

import os
import numpy as np

B, N_TOK, N_ATOMS = 4, 2048, 16384
C_TOKEN, C_ATOM, C_PAIR = 384, 128, 16
N_Q, N_K, N_HEADS, N_BLOCKS = 32, 128, 4, 3
DH = C_ATOM // N_HEADS

HALO = 256
OWN = N_ATOMS // 2
N_EXT = OWN + 2 * HALO          # 8704
PAD = (N_K - N_Q) // 2          # 48
NW_EXT = N_EXT // N_Q           # 272
SCALE = float(1.0 / np.sqrt(DH))

_CACHE = {}
_DBG = bool(os.environ.get("KERNEL_DEBUG_TIMING"))


def _masks():
    m = _CACHE.get("masks")
    if m is None:
        war = np.arange(NW_EXT)[:, None] * N_Q - PAD + np.arange(N_K)
        m = np.empty((8, NW_EXT, N_K), np.float32)
        for c in range(8):
            gs = (c % 2) * OWN - HALO
            kpos = gs + war
            m[c] = np.where((kpos >= 0) & (kpos < N_ATOMS), 0.0, -1e9)
        _CACHE["masks"] = m
    return m


def _shard_math_dev(np_, jax, x, bias, mask, Wq, Wk, Wv, Wo, Wt1, Wt2,
                    W_out):
    """One shard's 3 blocks + out proj. np_ is numpy or jax.numpy; LN affine
    is pre-folded into the weights by the caller."""
    jnp = np_
    nw = NW_EXT

    def windows(t):  # [N_EXT+2*PAD, C] -> [nw, N_K, C] (stride-32 slices)
        blocks = t.reshape(nw + 3, N_Q, C_ATOM)
        return jnp.concatenate([blocks[j:j + nw] for j in range(4)], axis=1)

    def ln(h):
        m = h.mean(-1, keepdims=True)
        v = h.var(-1, keepdims=True)
        if jax is None:
            return (h - m) / np_.sqrt(v + 1e-5)
        return (h - m) * jax.lax.rsqrt(v + 1e-5)

    def pad_kv(t):
        return jnp.pad(t, ((PAD, PAD), (0, 0)))

    for l in range(N_BLOCKS):
        h = ln(x)
        q = (h @ Wq[l]).reshape(nw, N_Q, N_HEADS, DH)
        k = windows(pad_kv(h @ Wk[l])).reshape(nw, N_K, N_HEADS, DH)
        v = windows(pad_kv(h @ Wv[l])).reshape(nw, N_K, N_HEADS, DH)
        s = jnp.einsum('wqhd,wkhd->whqk', q, k) * SCALE
        s = s + bias[None] + mask[:, None, None, :]
        if jax is None:
            s = s - s.max(-1, keepdims=True)
            e = np_.exp(s)
            attn = e / e.sum(-1, keepdims=True)
        else:
            attn = jax.nn.softmax(s, axis=-1)
        o = jnp.einsum('whqk,wkhd->wqhd', attn, v).reshape(N_EXT, C_ATOM)
        x = x + o @ Wo[l]
        h2 = ln(x)
        relu = (lambda t: np_.maximum(t, 0.0)) if jax is None else jax.nn.relu
        x = x + relu(h2 @ Wt1[l]) @ Wt2[l]
        if jax is not None:
            # keeps neuronx-cc's SBUF allocator from merging the blocks into
            # one constraint group, which ICEs (NCC_IIGCA118)
            x = jax.lax.optimization_barrier(x)

    return x @ W_out  # full ext length; host slices the owned range


def _get_fns(ws, bias_b, masks):
    """8 jitted shard fns (one per core) with weights + that core's bias and
    mask closed over as baked constants — runtime signature is just f(x)."""
    fp = tuple(float(w.flat[i]) for w in ws for i in (0, w.size // 2, -1))
    fp += (bias_b.tobytes()[:256], masks[0].tobytes()[:256], "bf16io-v2")
    ent = _CACHE.get("ent")
    if ent is not None and ent[0] == fp:
        return ent[1], ent[2]
    try:
        os.environ.setdefault("JAX_COMPILATION_CACHE_DIR",
                              os.path.expanduser("~/.jax_kernel_cache"))
        import jax
        try:
            jax.config.update("jax_compilation_cache_dir",
                              os.environ["JAX_COMPILATION_CACHE_DIR"])
        except Exception:
            pass
        devs = jax.devices()
        if len(devs) < 8:
            raise RuntimeError("need 8 cores")
        import jax.numpy as jnp

        fns = []
        for c in range(8):

            def f(x, _b=bias_b[c // 2], _m=masks[c]):
                # bf16 over the wire both ways (tunnel-bandwidth bound);
                # compute in fp32 on device
                x = x.astype(jnp.float32)
                r = _shard_math_dev(jnp, jax, x, _b, _m, *ws)
                return r.astype(jnp.bfloat16)

            fns.append(jax.jit(f))
        fn = (fns, (devs, jax))
    except Exception:
        fn = (None, (None, None))
    _CACHE["ent"] = (fp, fn[0], fn[1])
    return fn


def kernel(a, r_l, atom_to_token_idx, W_a, W_out, W_cl, W_cm, W_mlp1, W_mlp2,
           W_pb, Wq, Wk, Wv, Wo, ln1_g, ln1_b, Wt1, Wt2, ln2_g, ln2_b):
    import time as _t0mod
    _t_entry = _t0mod.perf_counter()
    a = np.asarray(a, np.float32)
    idx = np.asarray(atom_to_token_idx, np.int64)
    f32 = lambda w: np.asarray(w, np.float32)
    W_a, W_out, W_cl, W_cm, W_mlp1, W_mlp2, W_pb = map(
        f32, (W_a, W_out, W_cl, W_cm, W_mlp1, W_mlp2, W_pb))
    Wq, Wk, Wv, Wo, Wt1, Wt2 = map(f32, (Wq, Wk, Wv, Wo, Wt1, Wt2))
    ln1_g, ln1_b, ln2_g, ln2_b = map(f32, (ln1_g, ln1_b, ln2_g, ln2_b))

    # Fold LN affine into the following projections: (ln(x)*g+b) @ W
    # = ln(x) @ (g[:,None]*W) + b@W; the additive part rides on the residual
    # stream only through these matmuls, so add b@W as a bias via x-append?
    # Simpler: keep it exact by augmenting the weights with the bias folded
    # into an extra rank-1 update of the LN output is overkill here — instead
    # fold multiplicatively and add the bias row to the constant term by
    # shifting W: since ln output has zero mean per row, a constant bias
    # cannot be folded into W; handle b explicitly below.
    Wq_f = ln1_g[:, :, None] * Wq
    Wk_f = ln1_g[:, :, None] * Wk
    Wv_f = ln1_g[:, :, None] * Wv
    Wt1_f = ln2_g[:, :, None] * Wt1
    bq = np.einsum('lc,lcd->ld', ln1_b, Wq)
    bk = np.einsum('lc,lcd->ld', ln1_b, Wk)
    bv = np.einsum('lc,lcd->ld', ln1_b, Wv)
    bt1 = np.einsum('lc,lcd->ld', ln2_b, Wt1)
    if max(np.abs(x).max() for x in (bq, bk, bv, bt1)) > 0:
        # rare general case: keep biases by folding into an extra input col
        raise_bias = True
    else:
        raise_bias = False

    # Host: token projection + gather (cheap)
    a_tok = (a.reshape(-1, C_TOKEN) @ W_a).reshape(B, N_TOK, C_ATOM)

    # Pair bias (depends on first 128 atoms of each batch only)
    ab = np.take_along_axis(a_tok, idx[:, :N_K, None], axis=1)  # [B,128,C]
    p = ab @ (W_cl + W_cm)
    p = np.maximum(p, 0.0) @ W_mlp1
    p = np.maximum(p, 0.0) @ W_mlp2               # [B, 128, C_PAIR]
    g = p @ W_pb                                  # [B, 128, H]
    bias_b = g[:, :N_Q, None, :] + g[:, None, :N_K, :]  # [B, nq, nk, H]
    bias_b = np.ascontiguousarray(np.transpose(bias_b, (0, 3, 1, 2)))

    import time as _t
    t_prep = _t.perf_counter()
    masks = _masks()

    def build_x(c, src, dtype):
        # [N_EXT, C]: owned 8192 atoms + 256-atom halo each side; out-of-range
        # halo rows are zero.
        b, half = c // 2, c % 2
        x = np.zeros((N_EXT, C_ATOM), dtype)
        gs = half * OWN - HALO
        lo, hi = max(gs, 0), min(gs + N_EXT, N_ATOMS)
        x[lo - gs:hi - gs] = np.take(src[b], idx[b, lo:hi], axis=0)
        return x

    ws = (Wq_f, Wk_f, Wv_f, Wo, Wt1_f, Wt2, W_out)
    fns, (devs, jax) = (None, (None, None)) if raise_bias else _get_fns(
        ws, bias_b, masks)
    xs = None
    if fns is not None:
        try:
            from concurrent.futures import ThreadPoolExecutor

            import ml_dtypes
            bf16 = ml_dtypes.bfloat16
            a_tok_bf = a_tok.astype(bf16)
            outs = np.empty((8, OWN, C_ATOM), np.float32)

            def run_core(c):
                t0 = _t.perf_counter()
                x = build_x(c, a_tok_bf, bf16)
                t1 = _t.perf_counter()
                xd = jax.device_put(x, devs[c])
                fut = fns[c](xd)
                t2 = _t.perf_counter()
                outs[c] = np.asarray(fut)[HALO:HALO + OWN]
                t3 = _t.perf_counter()
                return t1 - t0, t2 - t1, t3 - t2

            with ThreadPoolExecutor(8) as ex:
                res = list(ex.map(run_core, range(8)))
            if _DBG:
                tt = np.array(res)
                print(f"[ktime] head={t_prep-_t_entry:.3f}s "
                      f"parallel_section={_t.perf_counter()-t_prep:.3f}s | "
                      f"thread-sums: gather={tt[:,0].sum():.3f} "
                      f"put+disp={tt[:,1].sum():.3f} "
                      f"wait+get={tt[:,2].sum():.3f}")
        except Exception:
            fns = None
    if fns is None:
        xs = np.stack([build_x(c, a_tok, np.float32) for c in range(8)])
        outs = np.stack([
            _np_shard_full(xs[c], bias_b[c // 2], masks[c], Wq_f, Wk_f, Wv_f,
                           Wo, Wt1_f, Wt2, W_out, bq, bk, bv, bt1)
            for c in range(8)])

    return outs.reshape(B, N_ATOMS, C_ATOM)


def _np_shard_full(x, bias, mask, Wq, Wk, Wv, Wo, Wt1, Wt2, W_out,
                   bq, bk, bv, bt1):
    key_idx = np.arange(NW_EXT)[:, None] * N_Q + np.arange(N_K)

    def ln(h):
        m = h.mean(-1, keepdims=True)
        v = h.var(-1, keepdims=True)
        return (h - m) / np.sqrt(v + 1e-5)

    for l in range(N_BLOCKS):
        h = ln(x)
        q = (h @ Wq[l] + bq[l]).reshape(NW_EXT, N_Q, N_HEADS, DH)
        kp = np.pad(h @ Wk[l] + bk[l], ((PAD, PAD), (0, 0)))
        vp = np.pad(h @ Wv[l] + bv[l], ((PAD, PAD), (0, 0)))
        k = kp[key_idx].reshape(NW_EXT, N_K, N_HEADS, DH)
        v = vp[key_idx].reshape(NW_EXT, N_K, N_HEADS, DH)
        s = np.einsum('wqhd,wkhd->whqk', q, k, optimize=True) * SCALE
        s = s + bias[None] + mask[:, None, None, :]
        s -= s.max(-1, keepdims=True)
        e = np.exp(s)
        attn = e / e.sum(-1, keepdims=True)
        o = np.einsum('whqk,wkhd->wqhd', attn, v, optimize=True)
        x = x + o.reshape(N_EXT, C_ATOM) @ Wo[l]
        h2 = ln(x)
        x = x + np.maximum(h2 @ Wt1[l] + bt1[l], 0.0) @ Wt2[l]
    return (x @ W_out)[HALO:HALO + OWN]

